# revision 35
# baseline (speedup 1.0000x reference)
"""CantorAttentionPlus Trainium2 kernel.

Sparse KNN attention (B=2, N=2048, DIM=1024, H=16, K=64) over 8 NeuronCores.
Sharding: data-parallel over batch x head-parallel (core c -> batch c//4,
heads 4*(c%4)..4*(c%4)+3). The routes table only depends on seq position, so
all cores share one sparsity structure.

Pipeline (all matmuls bf16; fp32 runs in slow fp32_mode=HIGH on the PE):
  B: QKV projection. Q,K land head-major ([dchan, n], weights stationary);
     V lands token-major ([n, dchan], x stationary) so the AV matmul needs
     no transposes. A ones column augments V so row 64 of the AV output is
     the softmax denominator.
  D: windowed attention per 256-query tile. Queries/keys are reordered by
     RCM on the routes graph, so each tile attends a narrow key window,
     split into 32-aligned segments within 128-key blocks:
        S.T[k,q] = K_seg.T @ Q      (TensorE, one MM per segment)
        P = exp(scale*S)            (ScalarE, PSUM -> bf16)
        Pm = P * mult_mask          (VectorE; mask in {0,1,2} handles
                                     out-of-route keys and self duplicates)
        out_aug += V_aug_seg.T @ Pm (TensorE, accumulated over segments)
     normalization: 1/den via fast DVE reciprocal, broadcast to 64
     partitions with a tiny selector matmul, multiplied on VectorE.
  E: output projection (partial y over this core's 4 heads), bf16 partials
     summed on host. b_out and the (exactly foldable) v-bias term are added
     on host; q/k biases are added on-device during PSUM evacuation.
"""

import numpy as np

B, N, DIM, H, D, K = 2, 2048, 1024, 16, 64, 64
QT = 256            # query tile
NQT = N // QT       # query tiles
HPC = 4             # heads per core
NCORES = 8
SCALE = 1.0 / np.sqrt(np.float32(D))
NB = 4              # token blocks of 512 for projections

_CACHE = {}


# ---------------------------------------------------------------- host prep

def _rcm_perm(routes: np.ndarray) -> np.ndarray:
    """Bandwidth-reducing ordering of the routes graph (symmetrized)."""
    try:
        from scipy.sparse import csr_matrix
        from scipy.sparse.csgraph import reverse_cuthill_mckee
        rows = np.repeat(np.arange(N), routes.shape[1])
        a = csr_matrix((np.ones(rows.size, np.float64), (rows, routes.ravel())),
                       shape=(N, N))
        a = ((a + a.T) > 0).astype(np.float64).tocsr()
        return np.asarray(reverse_cuthill_mckee(a, symmetric_mode=True),
                          dtype=np.int64)
    except Exception:
        return np.arange(N, dtype=np.int64)


def _segments_and_mask(routes: np.ndarray):
    """Permutation, per-tile key segments, packed multiplicity mask chunks.

    A segment is (tile, jblock, base, width): keys
    [128*jblock+base, 128*jblock+base+width) with base in {0,32,64} and
    width a multiple of 32 (<=128), so PSUM partition bases stay 32-aligned.
    """
    import ml_dtypes
    perm = _rcm_perm(routes)
    inv = np.empty(N, np.int64)
    inv[perm] = np.arange(N)
    rs = inv[routes[perm]]                      # (N, K) in sorted coords

    # multiplicity matrix M[key, query] (counts duplicates: self appears
    # twice when argpartition already included it), in log space pre-divided
    # by the softmax scale: exp(scale*(S + lm)) = mult * exp(scale*S)
    mt = np.zeros((N, N), np.float32)
    qidx = np.repeat(np.arange(N), routes.shape[1])
    np.add.at(mt, (rs.ravel(), qidx), 1.0)
    lm = np.where(mt > 0, np.log(np.maximum(mt, 1e-9)) * np.sqrt(np.float32(D)),
                  -1e6)
    mtb = lm.astype(ml_dtypes.bfloat16)

    segs = [[] for _ in range(NQT)]             # (jblock, base, width)
    for t in range(NQT):
        blk = rs[t * QT:(t + 1) * QT]
        a0 = (int(blk.min()) // 32) * 32
        a1 = min((int(blk.max()) + 32) // 32 * 32, N)
        # full 128-key blocks: keeps lhsT at 128 cols (Fast Weight Load on)
        # and all PSUM bases at 0; the mask zeroes the extra rows for free
        for j in range(a0 // 128, (a1 + 127) // 128):
            segs[t].append((j, 0, 128))

    mask_blocks = []
    for t in range(NQT):
        for (j, base, width) in segs[t]:
            mask_blocks.append(mtb[128 * j:128 * (j + 1),
                                   t * QT:(t + 1) * QT])
    mask = np.ascontiguousarray(np.stack(mask_blocks, 0))   # (S, 128, QT)
    return perm, segs, mask


# ------------------------------------------------------------- device program

def _build_program(segs, reps=1):
    import concourse.tile as tile
    import concourse.mybir as mybir
    from concourse import bacc
    from concourse.masks import make_identity

    f32 = mybir.dt.float32
    bf16 = mybir.dt.bfloat16
    Act = mybir.ActivationFunctionType
    nc = bacc.Bacc("TRN2", target_bir_lowering=False, debug=False,
                   num_devices=NCORES)

    nseg = sum(len(s) for s in segs)
    seg_off = np.cumsum([0] + [len(s) for s in segs])
    nsmax = max(len(s) for s in segs)

    # tile t is ready once projections cover its queries and all key blocks
    nb_req = []
    for t in range(NQT):
        jmax = max(j for (j, _, _) in segs[t])
        need_tok = max(128 * (jmax + 1), (t + 1) * QT)
        nb_req.append((need_tok + 511) // 512 - 1)

    xT = nc.dram_tensor("xT", [NB, 128, 8, 512], bf16,
                        kind="ExternalInput").ap()
    wqkT = nc.dram_tensor("wqkT", [128, 8, 4 * 128], bf16,
                          kind="ExternalInput").ap()
    wvT = nc.dram_tensor("wvT", [128, 8, 256], bf16,
                         kind="ExternalInput").ap()
    woT = nc.dram_tensor("woT", [128, 2, DIM], bf16,
                         kind="ExternalInput").ap()
    bqk = nc.dram_tensor("bqk", [128, 4], f32, kind="ExternalInput").ap()
    maskT = nc.dram_tensor("maskT", [nseg, 128, QT], bf16,
                           kind="ExternalInput").ap()
    yT = nc.dram_tensor("yT", [NB, 128, 8, 512], bf16,
                        kind="ExternalOutput").ap()

    with tile.TileContext(nc) as tc:
        with (
            tc.tile_pool(name="persist", bufs=1) as persist,
            tc.tile_pool(name="wpool", bufs=1) as wpool,
            tc.tile_pool(name="xpool", bufs=2) as xpool,
            tc.tile_pool(name="mpool", bufs=2) as mpool,
            tc.tile_pool(name="epool", bufs=4) as epool,
            tc.tile_pool(name="spool", bufs=2) as spool,
            tc.tile_pool(name="ypool", bufs=2) as ypool,
            # PSUM: 8 banks: proj/out-proj 2 + v/norm 2 + scores 2 + attnout 2
            tc.tile_pool(name="ps_a", bufs=2, space="PSUM") as ps_a,
            tc.tile_pool(name="ps_b", bufs=2, space="PSUM") as ps_b,
            tc.tile_pool(name="ps_sc", bufs=2, space="PSUM") as ps_sc,
            tc.tile_pool(name="ps_out", bufs=2, space="PSUM") as ps_out,
        ):
          for _rep in range(reps):
            # ---- persistent state
            wqk_sb = wpool.tile([128, 8, 4 * 128], bf16)
            wv_sb = wpool.tile([128, 8, 256], bf16)
            wo_sb = wpool.tile([128, 2, DIM], bf16)
            bqk_sb = wpool.tile([128, 4], f32)
            for c in range(4):
                nc.scalar.dma_start(out=wqk_sb[:, 2 * c:2 * c + 2],
                                    in_=wqkT[:, 2 * c:2 * c + 2])
            nc.scalar.dma_start(out=bqk_sb, in_=bqk)
            nc.scalar.dma_start(out=wv_sb, in_=wvT)
            nc.scalar.dma_start(out=wo_sb, in_=woT)

            qk_sb = persist.tile([128, 4, N], bf16)   # q0 q1 k0 k1
            v_sb = persist.tile([128, 16, HPC, D + 1], bf16)
            outn = persist.tile([128, 2, N], bf16)
            nc.vector.memset(v_sb[:, :, :, D], 1.0)

            itmp = wpool.tile([128, 128], f32, tag="itmp")
            make_identity(nc, itmp)
            ident_b = persist.tile([128, 128], bf16)
            nc.vector.tensor_copy(out=ident_b, in_=itmp)

            # sel2[:, P] broadcasts 1/den of heads (2P, 2P+1) onto output
            # partitions 0-63 / 64-127 in one matmul
            sel_f = wpool.tile([97, 2, 2 * D], f32, tag="self")
            nc.vector.memset(sel_f, 0.0)
            ones1 = wpool.tile([1, D], f32, tag="ones1")
            nc.vector.memset(ones1, 1.0)
            for h in range(HPC):
                nc.vector.tensor_copy(
                    out=sel_f[32 * h:32 * h + 1, h // 2,
                              (h % 2) * D:(h % 2) * D + D],
                    in_=ones1)
            sel2 = persist.tile([97, 2, 2 * D], bf16)
            nc.vector.tensor_copy(out=sel2, in_=sel_f)

            # ---- interleaved schedule
            def phase_b(nb):
                ncols = slice(nb * 512, (nb + 1) * 512)
                x_nb = xpool.tile([128, 8, 512], bf16, tag="xs")
                for c in range(4):
                    nc.sync.dma_start(out=x_nb[:, 2 * c:2 * c + 2],
                                      in_=xT[nb][:, 2 * c:2 * c + 2])
                for mb in range(4):
                    ps = ps_a.tile([128, 512], f32, tag="pj")
                    for ic in range(8):
                        nc.tensor.matmul(
                            ps, wqk_sb[:, ic, mb * 128:(mb + 1) * 128],
                            x_nb[:, ic], start=(ic == 0), stop=(ic == 7))
                    nc.vector.tensor_scalar_add(
                        out=qk_sb[:, mb, ncols], in0=ps,
                        scalar1=bqk_sb[:, mb:mb + 1])
                for tk in range(4):
                    psv = ps_b.tile([128, 256], f32, tag="pv")
                    for ic in range(8):
                        nc.tensor.matmul(
                            psv, x_nb[:, ic, tk * 128:(tk + 1) * 128],
                            wv_sb[:, ic], start=(ic == 0), stop=(ic == 7))
                    nc.vector.tensor_copy(
                        out=v_sb[:, 4 * nb + tk, :, 0:D],
                        in_=psv.rearrange("p (h d) -> p h d", h=HPC))

            def phase_d(t):
                """Emit tile t's attention; return a deferred norm emitter.

                Emission is software-pipelined: head h's score/mask matmuls
                are emitted before head h-1's AV matmuls, so the PE never
                sits behind an exp on the critical path. Segments are packed
                two per PSUM bank so one exp covers a pair.
                """
                tcols = slice(t * QT, (t + 1) * QT)
                sg = segs[t]
                ns = len(sg)
                i0 = int(seg_off[t])
                m_t = mpool.tile([128, nsmax, QT], bf16, tag="mask")
                nc.sync.dma_start(
                    out=m_t[:, 0:ns],
                    in_=maskT[i0:i0 + ns].rearrange("a p q -> p a q"))
                o4 = spool.tile([128, 2, QT], f32, tag="o4")
                den4 = spool.tile([97, QT], f32, tag="den4")
                nc.vector.memset(den4, 1.0)

                p_of = [[None] * ns for _ in range(HPC)]   # (h, si) -> AP
                po_h = [None] * HPC

                def emit_sm(h):
                    hp, hoff = h // 2, (h % 2) * 64
                    for s0 in range(0, ns, 2):
                        pair = sg[s0:s0 + 2]
                        ps2 = ps_sc.tile([128, 2 * QT], f32, tag="sc")
                        pb2 = epool.tile([128, 2 * QT], bf16, tag="psc")
                        for pi, (j, base, width) in enumerate(pair):
                            cc = slice(pi * QT, (pi + 1) * QT)
                            a0 = 128 * j + base
                            nc.tensor.matmul(
                                ps2[base:base + width, cc],
                                qk_sb[hoff:hoff + 64, 2 + hp, a0:a0 + width],
                                qk_sb[hoff:hoff + 64, 0 + hp, tcols],
                                start=True, stop=False)
                            nc.tensor.matmul(
                                ps2[base:base + width, cc],
                                ident_b[:, base:base + width],
                                m_t[:, s0 + pi],
                                start=False, stop=True, skip_group_check=True)
                        w = len(pair) * QT
                        nc.scalar.activation(
                            out=pb2[:, 0:w], in_=ps2[:, 0:w], func=Act.Exp,
                            scale=float(SCALE))
                        for pi in range(len(pair)):
                            p_of[h][s0 + pi] = pb2[:, pi * QT:(pi + 1) * QT]

                def emit_av(h):
                    hp, hoff = h // 2, (h % 2) * 64
                    po = ps_out.tile([65, QT], f32, tag="po")
                    po_h[h] = po
                    for si, (j, base, width) in enumerate(sg):
                        nc.tensor.matmul(
                            po, v_sb[base:base + width, j, h, :],
                            p_of[h][si][base:base + width],
                            start=(si == 0), stop=(si == ns - 1))
                    nc.vector.tensor_copy(out=o4[hoff:hoff + 64, hp],
                                          in_=po[0:D])
                    nc.vector.tensor_copy(out=den4[32 * h:32 * h + 1],
                                          in_=po[D:D + 1])

                for h in range(HPC):
                    emit_sm(h)
                    if h > 0:
                        emit_av(h - 1)
                emit_av(HPC - 1)

                def emit_norm():
                    rd4f = spool.tile([97, QT], f32, tag="rd4f")
                    rd4 = spool.tile([97, QT], bf16, tag="rd4")
                    nc.vector.reciprocal_approx_fast(out=rd4f, in_=den4)
                    nc.vector.tensor_copy(out=rd4, in_=rd4f)
                    for P in range(2):
                        psn = ps_b.tile([128, QT], f32, tag="pv")
                        nc.tensor.matmul(psn, sel2[:, P], rd4,
                                         start=True, stop=True)
                        nc.vector.tensor_mul(outn[:, P, tcols],
                                             o4[:, P], psn)
                return emit_norm

            def phase_e(nb):
                ncols = slice(nb * 512, (nb + 1) * 512)
                y_sb = ypool.tile([128, 8, 512], bf16, tag="ytile")
                for ob in range(8):
                    ps = ps_a.tile([128, 512], f32, tag="pj")
                    for hp in range(2):
                        nc.tensor.matmul(
                            ps, wo_sb[:, hp, ob * 128:(ob + 1) * 128],
                            outn[:, hp, ncols],
                            start=(hp == 0), stop=(hp == 1))
                    if ob % 2 == 0:
                        nc.scalar.copy(out=y_sb[:, ob], in_=ps)
                    else:
                        nc.vector.tensor_copy(out=y_sb[:, ob], in_=ps)
                    if ob % 2 == 1:
                        nc.sync.dma_start(out=yT[nb][:, ob - 1:ob + 1],
                                          in_=y_sb[:, ob - 1:ob + 1])

            done_d = 0
            done_e = 0
            done_norm = 0
            pending_norm = None

            def emit_d_with_lag(t):
                nonlocal pending_norm, done_norm
                nfn = phase_d(t)
                if pending_norm is not None:
                    pending_norm()
                    done_norm += 1
                pending_norm = nfn

            # interleave one projection block before each of the first four
            # attention tiles and output-projection blocks between the rest,
            # so the PE always has dense matmul work while exp chains drain
            # (emission order is a scheduling hint; deps keep it correct)
            for t in range(NQT):
                if t < NB:
                    phase_b(t)
                emit_d_with_lag(t)
                done_d += 1
                while done_e < NB and 2 * done_e + 1 < done_norm:
                    phase_e(done_e)
                    done_e += 1
            if pending_norm is not None:
                pending_norm()
                done_norm += 1
                pending_norm = None
            while done_e < NB:
                phase_e(done_e)
                done_e += 1

    nc.compile()
    return nc


# ------------------------------------------------------------------- kernel

def kernel(x, routes, w_qkv, b_qkv, w_out, b_out, _bench=None):
    import ml_dtypes
    bf = ml_dtypes.bfloat16
    x = np.asarray(x, np.float32)
    routes = np.asarray(routes, np.int32)
    w_qkv = np.asarray(w_qkv, np.float32)
    b_qkv = np.asarray(b_qkv, np.float32)
    w_out = np.asarray(w_out, np.float32)
    b_out = np.asarray(b_out, np.float32)

    from concourse.bass_utils import run_bass_kernel_spmd

    perm, segs, mask = _segments_and_mask(routes)

    key = tuple((t, tuple(s)) for t, s in enumerate(segs))
    if key not in _CACHE:
        _CACHE[key] = _build_program(segs)
    nc = _CACHE[key]

    # per-core inputs
    xs = x[:, perm, :]                                  # sorted tokens
    wq = w_qkv.reshape(3, H, D, DIM)
    bq = b_qkv.reshape(3, H, D)
    in_maps = []
    for c in range(NCORES):
        b, h0 = c // 4, HPC * (c % 4)
        # xT[nb][p, ic*512+col] = x[b, 512*nb+col, 128*ic+p]
        xT = np.ascontiguousarray(
            xs[b].T.reshape(8, 128, NB, 512).transpose(2, 1, 0, 3)).astype(bf)
        # q/k weight blocks: q(hp0) q(hp1) k(hp0) k(hp1); each 128 chans
        wblk, bblk = [], []
        for s in (0, 1):                                # q, k
            for hp in range(2):
                hh = h0 + 2 * hp
                wblk.append(wq[s, hh:hh + 2].reshape(128, DIM))
                bblk.append(bq[s, hh:hh + 2].reshape(128))
        wcat = np.stack(wblk, 0)                        # (4, 128, DIM)
        wqkT = np.ascontiguousarray(
            wcat.reshape(4, 128, 8, 128).transpose(3, 2, 0, 1)
                .reshape(128, 8, 4 * 128)).astype(bf)
        bqk_a = np.ascontiguousarray(np.stack(bblk, 1))  # (128, 4)
        # v weights, token-major: wvT[p, ic, n] = w_v[h0*D+n, 128*ic+p]
        wv_core = wq[2, h0:h0 + HPC].reshape(HPC * D, DIM)
        wvT = np.ascontiguousarray(
            wv_core.T.reshape(8, 128, 256).transpose(1, 0, 2)).astype(bf)
        woT = np.ascontiguousarray(
            w_out[:, h0 * D:(h0 + HPC) * D].T
                 .reshape(2, 128, DIM).transpose(1, 0, 2)).astype(bf)
        in_maps.append({"xT": xT, "wqkT": wqkT, "wvT": wvT, "bqk": bqk_a,
                        "maskT": mask, "woT": woT})

    res = run_bass_kernel_spmd(nc, in_maps, core_ids=list(range(NCORES)),
                               **(_bench or {}))

    # y partials: yT[nb][p, ob, col] = y[512*nb+col, 128*ob+p]
    y = np.zeros((B, N, DIM), np.float32)
    ys = np.zeros((B, N, DIM), np.float32)
    for c in range(NCORES):
        b = c // 4
        yc = res.results[c]["yT"].astype(np.float32)     # (NB,128,8,512)
        ys[b] += yc.transpose(0, 3, 2, 1).reshape(N, DIM)
    # b_out plus the exactly-foldable v-bias term (sum_k softmax = 1)
    bias = b_out + w_out @ b_qkv[2 * DIM:]
    for b in range(B):
        y[b, perm, :] = ys[b] + bias[None, :]
    if _bench is not None:
        kernel._last = res
    return y


# revision 47
# speedup vs baseline: 1.0863x; 1.0863x over previous
"""CantorAttentionPlus Trainium2 kernel.

Sparse KNN attention (B=2, N=2048, DIM=1024, H=16, K=64) over 8 NeuronCores.
Sharding: data-parallel over batch x head-parallel (core c -> batch c//4,
heads 4*(c%4)..4*(c%4)+3). The routes table only depends on seq position, so
all cores share one sparsity structure.

Pipeline (all matmuls bf16; fp32 runs in slow fp32_mode=HIGH on the PE and
fp8 fails the accuracy budget — zero-mean dot products keep fp8's ~7%
per-element error):
  B: QKV projection. Q,K land head-major ([dchan, n], weights stationary);
     V lands token-major ([n, dchan], x stationary) so the AV matmul needs
     no transposes. A ones column augments V so row 64 of the AV output is
     the softmax denominator.
  D: windowed attention per 256-query tile. Queries/keys are reordered by
     RCM on the routes graph, so each tile attends a narrow window of full
     128-key blocks (full blocks keep Fast Weight Load on and PSUM bases
     at partition 0):
        S.T[k,q] = K_blk.T @ Q       (TensorE, one MM per block, pairs of
                                      blocks share a PSUM bank)
        P = exp(scale*S)             (ScalarE, one exp per block pair)
        P *= mult_mask               (VectorE; mask in {0,1,2} kills
                                      out-of-route keys, counts self dups)
        out_aug += V_aug_blk.T @ P   (TensorE, accumulated over blocks)
     Normalization: 1/den via fast DVE reciprocal, broadcast two heads per
     selector matmul, multiplied on VectorE. Emission is software-pipelined
     (scores of head h before AVs of head h-1, norms lag one tile) so the
     PE never waits on an exp and HAM stays at K=8/8.
  E: output projection (partial y over this core's 4 heads), bf16 partials
     summed on host. b_out and the (exactly foldable) v-bias term are added
     on host; q/k biases are added on-device during PSUM evacuation.
"""

import numpy as np

B, N, DIM, H, D, K = 2, 2048, 1024, 16, 64, 64
QT = 256            # query tile
NQT = N // QT       # query tiles
HPC = 4             # heads per core
NCORES = 8
SCALE = 1.0 / np.sqrt(np.float32(D))
NB = 4              # token blocks of 512 for projections

_CACHE = {}


# ---------------------------------------------------------------- host prep

def _rcm_perm(routes: np.ndarray) -> np.ndarray:
    """Bandwidth-reducing ordering of the routes graph (symmetrized)."""
    try:
        from scipy.sparse import csr_matrix
        from scipy.sparse.csgraph import reverse_cuthill_mckee
        rows = np.repeat(np.arange(N), routes.shape[1])
        a = csr_matrix((np.ones(rows.size, np.float64), (rows, routes.ravel())),
                       shape=(N, N))
        a = ((a + a.T) > 0).astype(np.float64).tocsr()
        return np.asarray(reverse_cuthill_mckee(a, symmetric_mode=True),
                          dtype=np.int64)
    except Exception:
        return np.arange(N, dtype=np.int64)


def _segments_and_mask(routes: np.ndarray):
    """Permutation, per-tile key blocks, packed multiplicity mask chunks."""
    import ml_dtypes
    perm = _rcm_perm(routes)
    inv = np.empty(N, np.int64)
    inv[perm] = np.arange(N)
    rs = inv[routes[perm]]                      # (N, K) in sorted coords

    # multiplicity matrix M[key, query] (self appears twice when
    # argpartition already included it)
    mt = np.zeros((N, N), np.float32)
    qidx = np.repeat(np.arange(N), routes.shape[1])
    np.add.at(mt, (rs.ravel(), qidx), 1.0)
    mtb = mt.astype(ml_dtypes.bfloat16)

    segs = [[] for _ in range(NQT)]             # key block indices
    for t in range(NQT):
        blk = rs[t * QT:(t + 1) * QT]
        for j in range(int(blk.min()) // 128, int(blk.max()) // 128 + 1):
            segs[t].append(j)

    mask_blocks = []
    for t in range(NQT):
        for j in segs[t]:
            mask_blocks.append(mtb[128 * j:128 * (j + 1),
                                   t * QT:(t + 1) * QT])
    mask = np.ascontiguousarray(np.stack(mask_blocks, 0))   # (S, 128, QT)
    return perm, segs, mask


# ------------------------------------------------------------- device program

def _build_program(segs, reps=1):
    import concourse.tile as tile
    import concourse.mybir as mybir
    from concourse import bacc

    f32 = mybir.dt.float32
    bf16 = mybir.dt.bfloat16
    Act = mybir.ActivationFunctionType
    nc = bacc.Bacc("TRN2", target_bir_lowering=False, debug=False,
                   num_devices=NCORES)

    nseg = sum(len(s) for s in segs)
    seg_off = np.cumsum([0] + [len(s) for s in segs])
    nsmax = max(len(s) for s in segs)

    # tile t is ready once projections cover its queries and all key blocks
    nb_req = []
    for t in range(NQT):
        need_tok = max(128 * (max(segs[t]) + 1), (t + 1) * QT)
        nb_req.append((need_tok + 511) // 512 - 1)

    xT = nc.dram_tensor("xT", [NB, 128, 8, 512], bf16,
                        kind="ExternalInput").ap()
    wqkT = nc.dram_tensor("wqkT", [128, 8, 4 * 128], bf16,
                          kind="ExternalInput").ap()
    wvT = nc.dram_tensor("wvT", [128, 8, 256], bf16,
                         kind="ExternalInput").ap()
    woT = nc.dram_tensor("woT", [128, 2, DIM], bf16,
                         kind="ExternalInput").ap()
    bqk = nc.dram_tensor("bqk", [128, 4], f32, kind="ExternalInput").ap()
    maskT = nc.dram_tensor("maskT", [nseg, 128, QT], bf16,
                           kind="ExternalInput").ap()
    yT = nc.dram_tensor("yT", [NB, 128, 8, 512], bf16,
                        kind="ExternalOutput").ap()

    with tile.TileContext(nc) as tc:
        with (
            tc.tile_pool(name="persist", bufs=1) as persist,
            tc.tile_pool(name="wpool", bufs=1) as wpool,
            tc.tile_pool(name="xpool", bufs=2) as xpool,
            tc.tile_pool(name="mpool", bufs=2) as mpool,
            tc.tile_pool(name="epool", bufs=4) as epool,
            tc.tile_pool(name="spool", bufs=2) as spool,
            tc.tile_pool(name="ypool", bufs=2) as ypool,
            # PSUM: 8 banks: proj/out-proj 2 + v/norm 2 + scores 2 + attn 2
            tc.tile_pool(name="ps_a", bufs=2, space="PSUM") as ps_a,
            tc.tile_pool(name="ps_b", bufs=2, space="PSUM") as ps_b,
            tc.tile_pool(name="ps_sc", bufs=2, space="PSUM") as ps_sc,
            tc.tile_pool(name="ps_out", bufs=2, space="PSUM") as ps_out,
        ):
          for _rep in range(reps):
            # ---- persistent state
            wqk_sb = wpool.tile([128, 8, 4 * 128], bf16)
            wv_sb = wpool.tile([128, 8, 256], bf16)
            wo_sb = wpool.tile([128, 2, DIM], bf16)
            bqk_sb = wpool.tile([128, 4], f32)
            for c in range(4):
                nc.scalar.dma_start(out=wqk_sb[:, 2 * c:2 * c + 2],
                                    in_=wqkT[:, 2 * c:2 * c + 2])
            nc.scalar.dma_start(out=bqk_sb, in_=bqk)
            nc.scalar.dma_start(out=wv_sb, in_=wvT)
            nc.scalar.dma_start(out=wo_sb, in_=woT)

            qk_sb = persist.tile([128, 4, N], bf16)   # q0 q1 k0 k1
            v_sb = persist.tile([128, 16, HPC, D + 1], bf16)
            outn = persist.tile([128, 2, N], bf16)
            nc.vector.memset(v_sb[:, :, :, D], 1.0)

            # sel2[:, P] broadcasts 1/den of heads (2P, 2P+1) onto output
            # partitions 0-63 / 64-127 in one matmul
            sel_f = wpool.tile([97, 2, 2 * D], f32, tag="self")
            nc.vector.memset(sel_f, 0.0)
            ones1 = wpool.tile([1, D], f32, tag="ones1")
            nc.vector.memset(ones1, 1.0)
            for h in range(HPC):
                nc.vector.tensor_copy(
                    out=sel_f[32 * h:32 * h + 1, h // 2,
                              (h % 2) * D:(h % 2) * D + D],
                    in_=ones1)
            sel2 = persist.tile([97, 2, 2 * D], bf16)
            nc.vector.tensor_copy(out=sel2, in_=sel_f)

            def phase_b(nb):
                ncols = slice(nb * 512, (nb + 1) * 512)
                x_nb = xpool.tile([128, 8, 512], bf16, tag="xs")
                for c in range(4):
                    nc.sync.dma_start(out=x_nb[:, 2 * c:2 * c + 2],
                                      in_=xT[nb][:, 2 * c:2 * c + 2])
                for mb in range(4):
                    ps = ps_a.tile([128, 512], f32, tag="pj")
                    for ic in range(8):
                        nc.tensor.matmul(
                            ps, wqk_sb[:, ic, mb * 128:(mb + 1) * 128],
                            x_nb[:, ic], start=(ic == 0), stop=(ic == 7))
                    nc.scalar.activation(
                        out=qk_sb[:, mb, ncols], in_=ps, func=Act.Identity,
                        bias=bqk_sb[:, mb:mb + 1], scale=1.0)
                for tk in range(4):
                    psv = ps_b.tile([128, 256], f32, tag="pv")
                    for ic in range(8):
                        nc.tensor.matmul(
                            psv, x_nb[:, ic, tk * 128:(tk + 1) * 128],
                            wv_sb[:, ic], start=(ic == 0), stop=(ic == 7))
                    nc.vector.tensor_copy(
                        out=v_sb[:, 4 * nb + tk, :, 0:D],
                        in_=psv.rearrange("p (h d) -> p h d", h=HPC))

            def phase_d(t):
                """Emit tile t's attention; return a deferred norm emitter.

                Software-pipelined: head h's score matmuls are emitted
                before head h-1's AV matmuls so the PE never waits on an
                exp; key blocks are packed two per PSUM bank so one exp
                and one mask-multiply cover a pair.
                """
                tcols = slice(t * QT, (t + 1) * QT)
                sg = segs[t]
                ns = len(sg)
                i0 = int(seg_off[t])
                m_t = mpool.tile([128, nsmax, QT], bf16, tag="mask")
                nc.sync.dma_start(
                    out=m_t[:, 0:ns],
                    in_=maskT[i0:i0 + ns].rearrange("a p q -> p a q"))
                o4 = spool.tile([128, 2, QT], f32, tag="o4")
                den4 = spool.tile([97, QT], f32, tag="den4")
                nc.vector.memset(den4, 1.0)

                p_of = [[None] * ns for _ in range(HPC)]   # (h, si) -> AP

                def emit_sm(h):
                    hp, hoff = h // 2, (h % 2) * 64
                    for s0 in range(0, ns, 2):
                        npair = min(2, ns - s0)
                        w = npair * QT
                        ps2 = ps_sc.tile([128, 2 * QT], f32, tag="sc")
                        pb2 = epool.tile([128, 2 * QT], bf16, tag="psc")
                        for pi in range(npair):
                            j = sg[s0 + pi]
                            cc = slice(pi * QT, (pi + 1) * QT)
                            nc.tensor.matmul(
                                ps2[:, cc],
                                qk_sb[hoff:hoff + 64, 2 + hp,
                                      128 * j:128 * (j + 1)],
                                qk_sb[hoff:hoff + 64, 0 + hp, tcols],
                                start=True, stop=True)
                        nc.scalar.activation(
                            out=pb2[:, 0:w], in_=ps2[:, 0:w], func=Act.Exp,
                            scale=float(SCALE))
                        nc.vector.tensor_mul(
                            pb2[:, 0:w].rearrange("p (a q) -> p a q",
                                                  a=npair),
                            pb2[:, 0:w].rearrange("p (a q) -> p a q",
                                                  a=npair),
                            m_t[:, s0:s0 + npair])
                        for pi in range(npair):
                            p_of[h][s0 + pi] = pb2[:, pi * QT:(pi + 1) * QT]

                def emit_av(h):
                    hp, hoff = h // 2, (h % 2) * 64
                    po = ps_out.tile([65, QT], f32, tag="po")
                    for si, j in enumerate(sg):
                        nc.tensor.matmul(
                            po, v_sb[:, j, h, :], p_of[h][si],
                            start=(si == 0), stop=(si == ns - 1))
                    nc.vector.tensor_copy(out=o4[hoff:hoff + 64, hp],
                                          in_=po[0:D])
                    nc.vector.tensor_copy(out=den4[32 * h:32 * h + 1],
                                          in_=po[D:D + 1])

                for h in range(HPC):
                    emit_sm(h)
                    if h > 0:
                        emit_av(h - 1)
                emit_av(HPC - 1)

                def emit_norm():
                    rd4f = spool.tile([97, QT], f32, tag="rd4f")
                    rd4 = spool.tile([97, QT], bf16, tag="rd4")
                    nc.vector.reciprocal_approx_fast(out=rd4f, in_=den4)
                    nc.vector.tensor_copy(out=rd4, in_=rd4f)
                    for P in range(2):
                        psn = ps_b.tile([128, QT], f32, tag="pv")
                        nc.tensor.matmul(psn, sel2[:, P], rd4,
                                         start=True, stop=True)
                        nc.vector.tensor_mul(outn[:, P, tcols],
                                             o4[:, P], psn)
                return emit_norm

            def phase_e(nb):
                ncols = slice(nb * 512, (nb + 1) * 512)
                y_sb = ypool.tile([128, 8, 512], bf16, tag="ytile")
                for ob in range(8):
                    ps = ps_a.tile([128, 512], f32, tag="pj")
                    for hp in range(2):
                        nc.tensor.matmul(
                            ps, wo_sb[:, hp, ob * 128:(ob + 1) * 128],
                            outn[:, hp, ncols],
                            start=(hp == 0), stop=(hp == 1))
                    if ob % 2 == 0:
                        nc.scalar.copy(out=y_sb[:, ob], in_=ps)
                    else:
                        nc.vector.tensor_copy(out=y_sb[:, ob], in_=ps)
                    if ob % 2 == 1:
                        nc.sync.dma_start(out=yT[nb][:, ob - 1:ob + 1],
                                          in_=y_sb[:, ob - 1:ob + 1])

            done_e = 0
            done_norm = 0
            pending_norm = None

            def emit_d_with_lag(t):
                nonlocal pending_norm, done_norm
                nfn = phase_d(t)
                if pending_norm is not None:
                    pending_norm()
                    done_norm += 1
                pending_norm = nfn

            # interleave one projection block before each of the first four
            # attention tiles and output-projection blocks between the rest,
            # so the PE always has dense matmul work while exp chains drain
            # (emission order is a scheduling hint; deps keep it correct)
            for t in range(NQT):
                if t < NB:
                    phase_b(t)
                emit_d_with_lag(t)
                while done_e < NB and 2 * done_e + 1 < done_norm:
                    phase_e(done_e)
                    done_e += 1
            if pending_norm is not None:
                pending_norm()
                done_norm += 1
                pending_norm = None
            while done_e < NB:
                phase_e(done_e)
                done_e += 1

    nc.compile()
    return nc


# ------------------------------------------------------------------- kernel

def kernel(x, routes, w_qkv, b_qkv, w_out, b_out, _bench=None):
    import ml_dtypes
    bf = ml_dtypes.bfloat16
    x = np.asarray(x, np.float32)
    routes = np.asarray(routes, np.int32)
    w_qkv = np.asarray(w_qkv, np.float32)
    b_qkv = np.asarray(b_qkv, np.float32)
    w_out = np.asarray(w_out, np.float32)
    b_out = np.asarray(b_out, np.float32)

    from concourse.bass_utils import run_bass_kernel_spmd

    perm, segs, mask = _segments_and_mask(routes)

    key = tuple((t, tuple(s)) for t, s in enumerate(segs))
    if key not in _CACHE:
        _CACHE[key] = _build_program(segs)
    nc = _CACHE[key]

    # per-core inputs
    xs = x[:, perm, :]                                  # sorted tokens
    wq = w_qkv.reshape(3, H, D, DIM)
    bq = b_qkv.reshape(3, H, D)
    in_maps = []
    for c in range(NCORES):
        b, h0 = c // 4, HPC * (c % 4)
        # xT[nb][p, ic, col] = x[b, 512*nb+col, 128*ic+p]
        xT = np.ascontiguousarray(
            xs[b].T.reshape(8, 128, NB, 512).transpose(2, 1, 0, 3)).astype(bf)
        # q/k weight blocks: q(hp0) q(hp1) k(hp0) k(hp1); each 128 chans
        wblk, bblk = [], []
        for s in (0, 1):                                # q, k
            for hp in range(2):
                hh = h0 + 2 * hp
                wblk.append(wq[s, hh:hh + 2].reshape(128, DIM))
                bblk.append(bq[s, hh:hh + 2].reshape(128))
        wcat = np.stack(wblk, 0)                        # (4, 128, DIM)
        wqkT = np.ascontiguousarray(
            wcat.reshape(4, 128, 8, 128).transpose(3, 2, 0, 1)
                .reshape(128, 8, 4 * 128)).astype(bf)
        bqk_a = np.ascontiguousarray(np.stack(bblk, 1))  # (128, 4)
        # v weights, token-major: wvT[p, ic, n] = w_v[h0*D+n, 128*ic+p]
        wv_core = wq[2, h0:h0 + HPC].reshape(HPC * D, DIM)
        wvT = np.ascontiguousarray(
            wv_core.T.reshape(8, 128, 256).transpose(1, 0, 2)).astype(bf)
        woT = np.ascontiguousarray(
            w_out[:, h0 * D:(h0 + HPC) * D].T
                 .reshape(2, 128, DIM).transpose(1, 0, 2)).astype(bf)
        in_maps.append({"xT": xT, "wqkT": wqkT, "wvT": wvT, "bqk": bqk_a,
                        "maskT": mask, "woT": woT})

    res = run_bass_kernel_spmd(nc, in_maps, core_ids=list(range(NCORES)),
                               **(_bench or {}))

    # y partials: yT[nb][p, ob, col] = y[512*nb+col, 128*ob+p]
    y = np.zeros((B, N, DIM), np.float32)
    ys = np.zeros((B, N, DIM), np.float32)
    for c in range(NCORES):
        b = c // 4
        yc = res.results[c]["yT"].astype(np.float32)     # (NB,128,8,512)
        ys[b] += yc.transpose(0, 3, 2, 1).reshape(N, DIM)
    # b_out plus the exactly-foldable v-bias term (sum_k softmax = 1)
    bias = b_out + w_out @ b_qkv[2 * DIM:]
    for b in range(B):
        y[b, perm, :] = ys[b] + bias[None, :]
    if _bench is not None:
        kernel._last = res
    return y


# revision 52
# speedup vs baseline: 1.1126x; 1.0243x over previous
"""CantorAttentionPlus Trainium2 kernel.

Sparse KNN attention (B=2, N=2048, DIM=1024, H=16, K=64) over 8 NeuronCores.
Sharding: data-parallel over batch x head-parallel (core c -> batch c//4,
heads 4*(c%4)..4*(c%4)+3). The routes table only depends on seq position, so
all cores share one sparsity structure.

Pipeline (all matmuls bf16; fp32 runs in slow fp32_mode=HIGH on the PE and
fp8 fails the accuracy budget — zero-mean dot products keep fp8's ~7%
per-element error):
  B: QKV projection. Q,K land head-major ([dchan, n], weights stationary);
     V lands token-major ([n, dchan], x stationary) so the AV matmul needs
     no transposes. A ones column augments V so row 64 of the AV output is
     the softmax denominator.
  D: windowed attention per 256-query tile. Queries/keys are reordered by
     RCM on the routes graph, so each tile attends a narrow window of full
     128-key blocks (full blocks keep Fast Weight Load on and PSUM bases
     at partition 0):
        S.T[k,q] = K_blk.T @ Q       (TensorE, one MM per block, pairs of
                                      blocks share a PSUM bank)
        P = exp(scale*S)             (ScalarE, one exp per block pair)
        P *= mult_mask               (VectorE; mask in {0,1,2} kills
                                      out-of-route keys, counts self dups)
        out_aug += V_aug_blk.T @ P   (TensorE, accumulated over blocks)
     Normalization: 1/den via fast DVE reciprocal, broadcast two heads per
     selector matmul, multiplied on VectorE. Emission is software-pipelined
     (scores of head h before AVs of head h-1, norms lag one tile) so the
     PE never waits on an exp and HAM stays at K=8/8.
  E: output projection (partial y over this core's 4 heads), bf16 partials
     summed on host. b_out and the (exactly foldable) v-bias term are added
     on host; q/k biases are added on-device during PSUM evacuation.
"""

import numpy as np

B, N, DIM, H, D, K = 2, 2048, 1024, 16, 64, 64
QT = 256            # query tile
NQT = N // QT       # query tiles
HPC = 4             # heads per core
NCORES = 8
SCALE = 1.0 / np.sqrt(np.float32(D))
NB = 4              # token blocks of 512 for projections

_CACHE = {}


# ---------------------------------------------------------------- host prep

def _rcm_perm(routes: np.ndarray) -> np.ndarray:
    """Bandwidth-reducing ordering of the routes graph (symmetrized)."""
    try:
        from scipy.sparse import csr_matrix
        from scipy.sparse.csgraph import reverse_cuthill_mckee
        rows = np.repeat(np.arange(N), routes.shape[1])
        a = csr_matrix((np.ones(rows.size, np.float64), (rows, routes.ravel())),
                       shape=(N, N))
        a = ((a + a.T) > 0).astype(np.float64).tocsr()
        return np.asarray(reverse_cuthill_mckee(a, symmetric_mode=True),
                          dtype=np.int64)
    except Exception:
        return np.arange(N, dtype=np.int64)


def _segments_and_mask(routes: np.ndarray):
    """Permutation, per-tile key blocks, packed multiplicity mask chunks."""
    import ml_dtypes
    perm = _rcm_perm(routes)
    inv = np.empty(N, np.int64)
    inv[perm] = np.arange(N)
    rs = inv[routes[perm]]                      # (N, K) in sorted coords

    # multiplicity matrix M[key, query] (self appears twice when
    # argpartition already included it)
    mt = np.zeros((N, N), np.float32)
    qidx = np.repeat(np.arange(N), routes.shape[1])
    np.add.at(mt, (rs.ravel(), qidx), 1.0)
    mtb = mt.astype(ml_dtypes.bfloat16)

    segs = [[] for _ in range(NQT)]             # key block indices
    for t in range(NQT):
        blk = rs[t * QT:(t + 1) * QT]
        for j in range(int(blk.min()) // 128, int(blk.max()) // 128 + 1):
            segs[t].append(j)

    mask_blocks = []
    for t in range(NQT):
        for j in segs[t]:
            mask_blocks.append(mtb[128 * j:128 * (j + 1),
                                   t * QT:(t + 1) * QT])
    mask = np.ascontiguousarray(np.stack(mask_blocks, 0))   # (S, 128, QT)
    return perm, segs, mask


# ------------------------------------------------------------- device program

def _build_program(segs, reps=1):
    import concourse.tile as tile
    import concourse.mybir as mybir
    from concourse import bacc

    f32 = mybir.dt.float32
    bf16 = mybir.dt.bfloat16
    Act = mybir.ActivationFunctionType
    nc = bacc.Bacc("TRN2", target_bir_lowering=False, debug=False,
                   num_devices=NCORES)

    nseg = sum(len(s) for s in segs)
    seg_off = np.cumsum([0] + [len(s) for s in segs])
    nsmax = max(len(s) for s in segs)

    # tile t is ready once projections cover its queries and all key blocks
    nb_req = []
    for t in range(NQT):
        need_tok = max(128 * (max(segs[t]) + 1), (t + 1) * QT)
        nb_req.append((need_tok + 511) // 512 - 1)

    xT = nc.dram_tensor("xT", [NB, 128, 8, 512], bf16,
                        kind="ExternalInput").ap()
    wqkT = nc.dram_tensor("wqkT", [128, 8, 4 * 128], bf16,
                          kind="ExternalInput").ap()
    wvT = nc.dram_tensor("wvT", [128, 8, 256], bf16,
                         kind="ExternalInput").ap()
    woT = nc.dram_tensor("woT", [128, 2, DIM], bf16,
                         kind="ExternalInput").ap()
    bqk = nc.dram_tensor("bqk", [128, 4], f32, kind="ExternalInput").ap()
    maskT = nc.dram_tensor("maskT", [nseg, 128, QT], bf16,
                           kind="ExternalInput").ap()
    yT = nc.dram_tensor("yT", [NB, 128, 8, 512], bf16,
                        kind="ExternalOutput").ap()

    with tile.TileContext(nc) as tc:
        with (
            tc.tile_pool(name="persist", bufs=1) as persist,
            tc.tile_pool(name="wpool", bufs=1) as wpool,
            tc.tile_pool(name="xpool", bufs=2) as xpool,
            tc.tile_pool(name="mpool", bufs=2) as mpool,
            tc.tile_pool(name="epool", bufs=6) as epool,
            tc.tile_pool(name="spool", bufs=2) as spool,
            tc.tile_pool(name="ypool", bufs=2) as ypool,
            # PSUM: 8 banks: proj/out-proj 2 + v/norm 2 + scores 2 + attn 2
            tc.tile_pool(name="ps_a", bufs=2, space="PSUM") as ps_a,
            tc.tile_pool(name="ps_b", bufs=2, space="PSUM") as ps_b,
            tc.tile_pool(name="ps_sc", bufs=2, space="PSUM") as ps_sc,
            tc.tile_pool(name="ps_out", bufs=2, space="PSUM") as ps_out,
        ):
          for _rep in range(reps):
            # ---- persistent state
            wqk_sb = wpool.tile([128, 8, 4 * 128], bf16)
            wv_sb = wpool.tile([128, 8, 256], bf16)
            wo_sb = wpool.tile([128, 2, DIM], bf16)
            bqk_sb = wpool.tile([128, 4], f32)
            for c in range(4):
                nc.scalar.dma_start(out=wqk_sb[:, 2 * c:2 * c + 2],
                                    in_=wqkT[:, 2 * c:2 * c + 2])
            nc.scalar.dma_start(out=bqk_sb, in_=bqk)
            nc.scalar.dma_start(out=wv_sb, in_=wvT)
            nc.scalar.dma_start(out=wo_sb, in_=woT)

            qk_sb = persist.tile([128, 4, N], bf16)   # q0 q1 k0 k1
            v_sb = persist.tile([128, 16, HPC, D + 1], bf16)
            outn = persist.tile([128, 2, N], bf16)
            nc.vector.memset(v_sb[:, :, :, D], 1.0)

            # sel2[:, P] broadcasts 1/den of heads (2P, 2P+1) onto output
            # partitions 0-63 / 64-127 in one matmul
            sel_f = wpool.tile([97, 2, 2 * D], f32, tag="self")
            nc.vector.memset(sel_f, 0.0)
            ones1 = wpool.tile([1, D], f32, tag="ones1")
            nc.vector.memset(ones1, 1.0)
            for h in range(HPC):
                nc.vector.tensor_copy(
                    out=sel_f[32 * h:32 * h + 1, h // 2,
                              (h % 2) * D:(h % 2) * D + D],
                    in_=ones1)
            sel2 = persist.tile([97, 2, 2 * D], bf16)
            nc.vector.tensor_copy(out=sel2, in_=sel_f)

            def phase_b(nb):
                ncols = slice(nb * 512, (nb + 1) * 512)
                x_nb = xpool.tile([128, 8, 512], bf16, tag="xs")
                for c in range(4):
                    nc.sync.dma_start(out=x_nb[:, 2 * c:2 * c + 2],
                                      in_=xT[nb][:, 2 * c:2 * c + 2])
                for mb in range(4):
                    ps = ps_a.tile([128, 512], f32, tag="pj")
                    for ic in range(8):
                        nc.tensor.matmul(
                            ps, wqk_sb[:, ic, mb * 128:(mb + 1) * 128],
                            x_nb[:, ic], start=(ic == 0), stop=(ic == 7))
                    nc.scalar.activation(
                        out=qk_sb[:, mb, ncols], in_=ps, func=Act.Identity,
                        bias=bqk_sb[:, mb:mb + 1], scale=1.0)
                for tk in range(4):
                    psv = ps_b.tile([128, 256], f32, tag="pv")
                    for ic in range(8):
                        nc.tensor.matmul(
                            psv, x_nb[:, ic, tk * 128:(tk + 1) * 128],
                            wv_sb[:, ic], start=(ic == 0), stop=(ic == 7))
                    nc.vector.tensor_copy(
                        out=v_sb[:, 4 * nb + tk, :, 0:D],
                        in_=psv.rearrange("p (h d) -> p h d", h=HPC))

            def phase_d(t):
                """Emit tile t's attention; return a deferred norm emitter.

                Software-pipelined: head h's score matmuls are emitted
                before head h-1's AV matmuls so the PE never waits on an
                exp; key blocks are packed two per PSUM bank so one exp
                and one mask-multiply cover a pair.
                """
                tcols = slice(t * QT, (t + 1) * QT)
                sg = segs[t]
                ns = len(sg)
                i0 = int(seg_off[t])
                m_t = mpool.tile([128, nsmax, QT], bf16, tag="mask")
                nc.sync.dma_start(
                    out=m_t[:, 0:ns],
                    in_=maskT[i0:i0 + ns].rearrange("a p q -> p a q"))
                o4 = spool.tile([128, 2, QT], f32, tag="o4")
                den4 = spool.tile([97, QT], f32, tag="den4")
                nc.vector.memset(den4, 1.0)

                p_of = [[None] * ns for _ in range(HPC)]   # (h, si) -> AP

                def emit_sm(h):
                    hp, hoff = h // 2, (h % 2) * 64
                    for s0 in range(0, ns, 2):
                        npair = min(2, ns - s0)
                        w = npair * QT
                        ps2 = ps_sc.tile([128, 2 * QT], f32, tag="sc")
                        pb2 = epool.tile([128, 2 * QT], bf16, tag="psc")
                        for pi in range(npair):
                            j = sg[s0 + pi]
                            cc = slice(pi * QT, (pi + 1) * QT)
                            nc.tensor.matmul(
                                ps2[:, cc],
                                qk_sb[hoff:hoff + 64, 2 + hp,
                                      128 * j:128 * (j + 1)],
                                qk_sb[hoff:hoff + 64, 0 + hp, tcols],
                                start=True, stop=True)
                        nc.scalar.activation(
                            out=pb2[:, 0:w], in_=ps2[:, 0:w], func=Act.Exp,
                            scale=float(SCALE))
                        nc.vector.tensor_mul(
                            pb2[:, 0:w].rearrange("p (a q) -> p a q",
                                                  a=npair),
                            pb2[:, 0:w].rearrange("p (a q) -> p a q",
                                                  a=npair),
                            m_t[:, s0:s0 + npair])
                        for pi in range(npair):
                            p_of[h][s0 + pi] = pb2[:, pi * QT:(pi + 1) * QT]

                def emit_av(h):
                    hp, hoff = h // 2, (h % 2) * 64
                    po = ps_out.tile([65, QT], f32, tag="po")
                    for si, j in enumerate(sg):
                        nc.tensor.matmul(
                            po, v_sb[:, j, h, :], p_of[h][si],
                            start=(si == 0), stop=(si == ns - 1))
                    nc.vector.tensor_copy(out=o4[hoff:hoff + 64, hp],
                                          in_=po[0:D])
                    nc.vector.tensor_copy(out=den4[32 * h:32 * h + 1],
                                          in_=po[D:D + 1])

                for h in range(HPC):
                    emit_sm(h)
                    if h > 1:
                        emit_av(h - 2)
                emit_av(HPC - 2)
                emit_av(HPC - 1)

                def emit_norm():
                    rd4f = spool.tile([97, QT], f32, tag="rd4f")
                    rd4 = spool.tile([97, QT], bf16, tag="rd4")
                    nc.vector.reciprocal_approx_fast(out=rd4f, in_=den4)
                    nc.vector.tensor_copy(out=rd4, in_=rd4f)
                    for P in range(2):
                        psn = ps_b.tile([128, QT], f32, tag="pv")
                        nc.tensor.matmul(psn, sel2[:, P], rd4,
                                         start=True, stop=True)
                        nc.vector.tensor_mul(outn[:, P, tcols],
                                             o4[:, P], psn)
                return emit_norm

            y_tiles = {}

            def phase_e(nb, half):
                ncols = slice(nb * 512, (nb + 1) * 512)
                if half == 0:
                    y_new = ypool.tile([128, 8, 512], bf16, tag="ytile")
                    y_tiles[nb] = y_new
                y_sb = y_tiles[nb]
                for ob in range(4 * half, 4 * half + 4):
                    ps = ps_a.tile([128, 512], f32, tag="pj")
                    for hp in range(2):
                        nc.tensor.matmul(
                            ps, wo_sb[:, hp, ob * 128:(ob + 1) * 128],
                            outn[:, hp, ncols],
                            start=(hp == 0), stop=(hp == 1))
                    if ob % 2 == 0:
                        nc.scalar.copy(out=y_sb[:, ob], in_=ps)
                    else:
                        nc.vector.tensor_copy(out=y_sb[:, ob], in_=ps)
                    if ob % 2 == 1:
                        nc.sync.dma_start(out=yT[nb][:, ob - 1:ob + 1],
                                          in_=y_sb[:, ob - 1:ob + 1])

            done_e = 0
            done_norm = 0
            pending_norm = None

            def emit_d_with_lag(t):
                nonlocal pending_norm, done_norm
                nfn = phase_d(t)
                if pending_norm is not None:
                    pending_norm()
                    done_norm += 1
                pending_norm = nfn

            # interleave one projection block before each of the first four
            # attention tiles and output-projection blocks between the rest,
            # so the PE always has dense matmul work while exp chains drain
            # (emission order is a scheduling hint; deps keep it correct)
            for t in range(NQT):
                if t < NB:
                    phase_b(t)
                emit_d_with_lag(t)
                while done_e < 2 * NB and done_e + 1 < done_norm:
                    phase_e(done_e // 2, done_e % 2)
                    done_e += 1
            if pending_norm is not None:
                pending_norm()
                done_norm += 1
                pending_norm = None
            while done_e < 2 * NB:
                phase_e(done_e // 2, done_e % 2)
                done_e += 1

    nc.compile()
    return nc


# ------------------------------------------------------------------- kernel

def kernel(x, routes, w_qkv, b_qkv, w_out, b_out, _bench=None):
    import ml_dtypes
    bf = ml_dtypes.bfloat16
    x = np.asarray(x, np.float32)
    routes = np.asarray(routes, np.int32)
    w_qkv = np.asarray(w_qkv, np.float32)
    b_qkv = np.asarray(b_qkv, np.float32)
    w_out = np.asarray(w_out, np.float32)
    b_out = np.asarray(b_out, np.float32)

    from concourse.bass_utils import run_bass_kernel_spmd

    perm, segs, mask = _segments_and_mask(routes)

    key = tuple((t, tuple(s)) for t, s in enumerate(segs))
    if key not in _CACHE:
        _CACHE[key] = _build_program(segs)
    nc = _CACHE[key]

    # per-core inputs
    xs = x[:, perm, :]                                  # sorted tokens
    wq = w_qkv.reshape(3, H, D, DIM)
    bq = b_qkv.reshape(3, H, D)
    in_maps = []
    for c in range(NCORES):
        b, h0 = c // 4, HPC * (c % 4)
        # xT[nb][p, ic, col] = x[b, 512*nb+col, 128*ic+p]
        xT = np.ascontiguousarray(
            xs[b].T.reshape(8, 128, NB, 512).transpose(2, 1, 0, 3)).astype(bf)
        # q/k weight blocks: q(hp0) q(hp1) k(hp0) k(hp1); each 128 chans
        wblk, bblk = [], []
        for s in (0, 1):                                # q, k
            for hp in range(2):
                hh = h0 + 2 * hp
                wblk.append(wq[s, hh:hh + 2].reshape(128, DIM))
                bblk.append(bq[s, hh:hh + 2].reshape(128))
        wcat = np.stack(wblk, 0)                        # (4, 128, DIM)
        wqkT = np.ascontiguousarray(
            wcat.reshape(4, 128, 8, 128).transpose(3, 2, 0, 1)
                .reshape(128, 8, 4 * 128)).astype(bf)
        bqk_a = np.ascontiguousarray(np.stack(bblk, 1))  # (128, 4)
        # v weights, token-major: wvT[p, ic, n] = w_v[h0*D+n, 128*ic+p]
        wv_core = wq[2, h0:h0 + HPC].reshape(HPC * D, DIM)
        wvT = np.ascontiguousarray(
            wv_core.T.reshape(8, 128, 256).transpose(1, 0, 2)).astype(bf)
        woT = np.ascontiguousarray(
            w_out[:, h0 * D:(h0 + HPC) * D].T
                 .reshape(2, 128, DIM).transpose(1, 0, 2)).astype(bf)
        in_maps.append({"xT": xT, "wqkT": wqkT, "wvT": wvT, "bqk": bqk_a,
                        "maskT": mask, "woT": woT})

    res = run_bass_kernel_spmd(nc, in_maps, core_ids=list(range(NCORES)),
                               **(_bench or {}))

    # y partials: yT[nb][p, ob, col] = y[512*nb+col, 128*ob+p]
    y = np.zeros((B, N, DIM), np.float32)
    ys = np.zeros((B, N, DIM), np.float32)
    for c in range(NCORES):
        b = c // 4
        yc = res.results[c]["yT"].astype(np.float32)     # (NB,128,8,512)
        ys[b] += yc.transpose(0, 3, 2, 1).reshape(N, DIM)
    # b_out plus the exactly-foldable v-bias term (sum_k softmax = 1)
    bias = b_out + w_out @ b_qkv[2 * DIM:]
    for b in range(B):
        y[b, perm, :] = ys[b] + bias[None, :]
    if _bench is not None:
        kernel._last = res
    return y


# revision 55
# speedup vs baseline: 1.1219x; 1.0083x over previous
"""CantorAttentionPlus Trainium2 kernel.

Sparse KNN attention (B=2, N=2048, DIM=1024, H=16, K=64) over 8 NeuronCores.
Sharding: data-parallel over batch x head-parallel (core c -> batch c//4,
heads 4*(c%4)..4*(c%4)+3). The routes table only depends on seq position, so
all cores share one sparsity structure.

Pipeline (all matmuls bf16; fp32 runs in slow fp32_mode=HIGH on the PE and
fp8 fails the accuracy budget — zero-mean dot products keep fp8's ~7%
per-element error):
  B: QKV projection. Q,K land head-major ([dchan, n], weights stationary);
     V lands token-major ([n, dchan], x stationary) so the AV matmul needs
     no transposes. A ones column augments V so row 64 of the AV output is
     the softmax denominator.
  D: windowed attention per 256-query tile. Queries/keys are reordered by
     RCM on the routes graph, so each tile attends a narrow window of full
     128-key blocks (full blocks keep Fast Weight Load on and PSUM bases
     at partition 0):
        S.T[k,q] = K_blk.T @ Q       (TensorE, one MM per block, pairs of
                                      blocks share a PSUM bank)
        P = exp(scale*S)             (ScalarE, one exp per block pair)
        P *= mult_mask               (VectorE; mask in {0,1,2} kills
                                      out-of-route keys, counts self dups)
        out_aug += V_aug_blk.T @ P   (TensorE, accumulated over blocks)
     Normalization: 1/den via fast DVE reciprocal, broadcast two heads per
     selector matmul, multiplied on VectorE. Emission is software-pipelined
     (scores of head h before AVs of head h-1, norms lag one tile) so the
     PE never waits on an exp and HAM stays at K=8/8.
  E: output projection (partial y over this core's 4 heads), bf16 partials
     summed on host. b_out and the (exactly foldable) v-bias term are added
     on host; q/k biases are added on-device during PSUM evacuation.
"""

import numpy as np

B, N, DIM, H, D, K = 2, 2048, 1024, 16, 64, 64
QT = 256            # query tile
NQT = N // QT       # query tiles
HPC = 4             # heads per core
NCORES = 8
SCALE = 1.0 / np.sqrt(np.float32(D))
NB = 4              # token blocks of 512 for projections

_CACHE = {}


# ---------------------------------------------------------------- host prep

def _rcm_perm(routes: np.ndarray) -> np.ndarray:
    """Bandwidth-reducing ordering of the routes graph (symmetrized)."""
    try:
        from scipy.sparse import csr_matrix
        from scipy.sparse.csgraph import reverse_cuthill_mckee
        rows = np.repeat(np.arange(N), routes.shape[1])
        a = csr_matrix((np.ones(rows.size, np.float64), (rows, routes.ravel())),
                       shape=(N, N))
        a = ((a + a.T) > 0).astype(np.float64).tocsr()
        return np.asarray(reverse_cuthill_mckee(a, symmetric_mode=True),
                          dtype=np.int64)
    except Exception:
        return np.arange(N, dtype=np.int64)


def _segments_and_mask(routes: np.ndarray):
    """Permutation, per-tile key blocks, packed multiplicity mask chunks."""
    import ml_dtypes
    perm = _rcm_perm(routes)
    inv = np.empty(N, np.int64)
    inv[perm] = np.arange(N)
    rs = inv[routes[perm]]                      # (N, K) in sorted coords

    # multiplicity matrix M[key, query] (self appears twice when
    # argpartition already included it)
    mt = np.zeros((N, N), np.float32)
    qidx = np.repeat(np.arange(N), routes.shape[1])
    np.add.at(mt, (rs.ravel(), qidx), 1.0)
    mtb = mt.astype(ml_dtypes.bfloat16)

    segs = [[] for _ in range(NQT)]             # key block indices
    for t in range(NQT):
        blk = rs[t * QT:(t + 1) * QT]
        for j in range(int(blk.min()) // 128, int(blk.max()) // 128 + 1):
            segs[t].append(j)

    mask_blocks = []
    for t in range(NQT):
        for j in segs[t]:
            mask_blocks.append(mtb[128 * j:128 * (j + 1),
                                   t * QT:(t + 1) * QT])
    mask = np.ascontiguousarray(np.stack(mask_blocks, 0))   # (S, 128, QT)
    return perm, segs, mask


# ------------------------------------------------------------- device program

def _build_program(segs, reps=1):
    import concourse.tile as tile
    import concourse.mybir as mybir
    from concourse import bacc

    f32 = mybir.dt.float32
    bf16 = mybir.dt.bfloat16
    Act = mybir.ActivationFunctionType
    nc = bacc.Bacc("TRN2", target_bir_lowering=False, debug=False,
                   num_devices=NCORES)

    nseg = sum(len(s) for s in segs)
    seg_off = np.cumsum([0] + [len(s) for s in segs])
    nsmax = max(len(s) for s in segs)

    # tile t is ready once projections cover its queries and all key blocks
    nb_req = []
    for t in range(NQT):
        need_tok = max(128 * (max(segs[t]) + 1), (t + 1) * QT)
        nb_req.append((need_tok + 511) // 512 - 1)

    xT = nc.dram_tensor("xT", [NB, 128, 8, 512], bf16,
                        kind="ExternalInput").ap()
    wqkT = nc.dram_tensor("wqkT", [128, 8, 4 * 128], bf16,
                          kind="ExternalInput").ap()
    wvT = nc.dram_tensor("wvT", [128, 8, 256], bf16,
                         kind="ExternalInput").ap()
    woT = nc.dram_tensor("woT", [128, 2, DIM], bf16,
                         kind="ExternalInput").ap()
    bqk = nc.dram_tensor("bqk", [128, 4], f32, kind="ExternalInput").ap()
    maskT = nc.dram_tensor("maskT", [nseg, 128, QT], bf16,
                           kind="ExternalInput").ap()
    yT = nc.dram_tensor("yT", [NB, 128, 8, 512], bf16,
                        kind="ExternalOutput").ap()

    with tile.TileContext(nc) as tc:
        with (
            tc.tile_pool(name="persist", bufs=1) as persist,
            tc.tile_pool(name="wpool", bufs=1) as wpool,
            tc.tile_pool(name="xpool", bufs=2) as xpool,
            tc.tile_pool(name="mpool", bufs=2) as mpool,
            tc.tile_pool(name="epool", bufs=8) as epool,
            tc.tile_pool(name="spool", bufs=2) as spool,
            tc.tile_pool(name="ypool", bufs=2) as ypool,
            # PSUM: 8 banks: proj/out-proj 2 + v/norm 2 + scores 2 + attn 2
            tc.tile_pool(name="ps_a", bufs=2, space="PSUM") as ps_a,
            tc.tile_pool(name="ps_b", bufs=2, space="PSUM") as ps_b,
            tc.tile_pool(name="ps_sc", bufs=2, space="PSUM") as ps_sc,
            tc.tile_pool(name="ps_out", bufs=2, space="PSUM") as ps_out,
        ):
          for _rep in range(reps):
            # ---- persistent state
            wqk_sb = wpool.tile([128, 8, 4 * 128], bf16)
            wv_sb = wpool.tile([128, 8, 256], bf16)
            wo_sb = wpool.tile([128, 2, DIM], bf16)
            bqk_sb = wpool.tile([128, 4], f32)
            for c in range(4):
                nc.scalar.dma_start(out=wqk_sb[:, 2 * c:2 * c + 2],
                                    in_=wqkT[:, 2 * c:2 * c + 2])
            nc.scalar.dma_start(out=bqk_sb, in_=bqk)
            nc.scalar.dma_start(out=wv_sb, in_=wvT)
            nc.scalar.dma_start(out=wo_sb, in_=woT)

            qk_sb = persist.tile([128, 4, N], bf16)   # q0 q1 k0 k1
            v_sb = persist.tile([128, 16, HPC, D + 1], bf16)
            outn = persist.tile([128, 2, N], bf16)
            nc.vector.memset(v_sb[:, :, :, D], 1.0)

            # sel2[:, P] broadcasts 1/den of heads (2P, 2P+1) onto output
            # partitions 0-63 / 64-127 in one matmul
            sel_f = wpool.tile([97, 2, 2 * D], f32, tag="self")
            nc.vector.memset(sel_f, 0.0)
            ones1 = wpool.tile([1, D], f32, tag="ones1")
            nc.vector.memset(ones1, 1.0)
            for h in range(HPC):
                nc.vector.tensor_copy(
                    out=sel_f[32 * h:32 * h + 1, h // 2,
                              (h % 2) * D:(h % 2) * D + D],
                    in_=ones1)
            sel2 = persist.tile([97, 2, 2 * D], bf16)
            nc.vector.tensor_copy(out=sel2, in_=sel_f)

            def phase_b(nb):
                ncols = slice(nb * 512, (nb + 1) * 512)
                x_nb = xpool.tile([128, 8, 512], bf16, tag="xs")
                for c in range(4):
                    nc.sync.dma_start(out=x_nb[:, 2 * c:2 * c + 2],
                                      in_=xT[nb][:, 2 * c:2 * c + 2])
                for mb in range(4):
                    ps = ps_a.tile([128, 512], f32, tag="pj")
                    for ic in range(8):
                        nc.tensor.matmul(
                            ps, wqk_sb[:, ic, mb * 128:(mb + 1) * 128],
                            x_nb[:, ic], start=(ic == 0), stop=(ic == 7))
                    nc.scalar.activation(
                        out=qk_sb[:, mb, ncols], in_=ps, func=Act.Identity,
                        bias=bqk_sb[:, mb:mb + 1], scale=1.0)
                for tk in range(4):
                    psv = ps_b.tile([128, 256], f32, tag="pv")
                    for ic in range(8):
                        nc.tensor.matmul(
                            psv, x_nb[:, ic, tk * 128:(tk + 1) * 128],
                            wv_sb[:, ic], start=(ic == 0), stop=(ic == 7))
                    nc.vector.tensor_copy(
                        out=v_sb[:, 4 * nb + tk, :, 0:D],
                        in_=psv.rearrange("p (h d) -> p h d", h=HPC))

            def phase_d(t):
                """Emit tile t's attention; return a deferred norm emitter.

                Software-pipelined: head h's score matmuls are emitted
                before head h-1's AV matmuls so the PE never waits on an
                exp; key blocks are packed two per PSUM bank so one exp
                and one mask-multiply cover a pair.
                """
                tcols = slice(t * QT, (t + 1) * QT)
                sg = segs[t]
                ns = len(sg)
                i0 = int(seg_off[t])
                m_t = mpool.tile([128, nsmax, QT], bf16, tag="mask")
                nc.sync.dma_start(
                    out=m_t[:, 0:ns],
                    in_=maskT[i0:i0 + ns].rearrange("a p q -> p a q"))
                o4 = spool.tile([128, 2, QT], f32, tag="o4")
                den4 = spool.tile([97, QT], f32, tag="den4")
                nc.vector.memset(den4, 1.0)

                p_of = [[None] * ns for _ in range(HPC)]   # (h, si) -> AP

                def emit_sm_pair(hp):
                    # interleave the two heads' score matmuls: adjacent MMs
                    # hit different 64-row groups of the PE array (64x128
                    # tiling) and overlap in hardware
                    for s0 in range(0, ns, 2):
                        npair = min(2, ns - s0)
                        w = npair * QT
                        hts = []
                        for h in (2 * hp, 2 * hp + 1):
                            ps2 = ps_sc.tile([128, 2 * QT], f32, tag="sc")
                            pb2 = epool.tile([128, 2 * QT], bf16, tag="psc")
                            hts.append((h, ps2, pb2))
                        for pi in range(npair):
                            j = sg[s0 + pi]
                            cc = slice(pi * QT, (pi + 1) * QT)
                            for (h, ps2, pb2) in hts:
                                hoff = (h % 2) * 64
                                nc.tensor.matmul(
                                    ps2[:, cc],
                                    qk_sb[hoff:hoff + 64, 2 + hp,
                                          128 * j:128 * (j + 1)],
                                    qk_sb[hoff:hoff + 64, 0 + hp, tcols],
                                    start=True, stop=True)
                        for (h, ps2, pb2) in hts:
                            nc.scalar.activation(
                                out=pb2[:, 0:w], in_=ps2[:, 0:w],
                                func=Act.Exp, scale=float(SCALE))
                            nc.vector.tensor_mul(
                                pb2[:, 0:w].rearrange("p (a q) -> p a q",
                                                      a=npair),
                                pb2[:, 0:w].rearrange("p (a q) -> p a q",
                                                      a=npair),
                                m_t[:, s0:s0 + npair])
                            for pi in range(npair):
                                p_of[h][s0 + pi] = pb2[:,
                                                       pi * QT:(pi + 1) * QT]

                def emit_av(h):
                    hp, hoff = h // 2, (h % 2) * 64
                    po = ps_out.tile([65, QT], f32, tag="po")
                    for si, j in enumerate(sg):
                        nc.tensor.matmul(
                            po, v_sb[:, j, h, :], p_of[h][si],
                            start=(si == 0), stop=(si == ns - 1))
                    nc.vector.tensor_copy(out=o4[hoff:hoff + 64, hp],
                                          in_=po[0:D])
                    nc.vector.tensor_copy(out=den4[32 * h:32 * h + 1],
                                          in_=po[D:D + 1])

                emit_sm_pair(0)
                emit_sm_pair(1)
                for h in range(HPC):
                    emit_av(h)

                def emit_norm():
                    rd4f = spool.tile([97, QT], f32, tag="rd4f")
                    rd4 = spool.tile([97, QT], bf16, tag="rd4")
                    nc.vector.reciprocal_approx_fast(out=rd4f, in_=den4)
                    nc.vector.tensor_copy(out=rd4, in_=rd4f)
                    for P in range(2):
                        psn = ps_b.tile([128, QT], f32, tag="pv")
                        nc.tensor.matmul(psn, sel2[:, P], rd4,
                                         start=True, stop=True)
                        nc.vector.tensor_mul(outn[:, P, tcols],
                                             o4[:, P], psn)
                return emit_norm

            y_tiles = {}

            def phase_e(nb, half):
                ncols = slice(nb * 512, (nb + 1) * 512)
                if half == 0:
                    y_new = ypool.tile([128, 8, 512], bf16, tag="ytile")
                    y_tiles[nb] = y_new
                y_sb = y_tiles[nb]
                for ob in range(4 * half, 4 * half + 4):
                    ps = ps_a.tile([128, 512], f32, tag="pj")
                    for hp in range(2):
                        nc.tensor.matmul(
                            ps, wo_sb[:, hp, ob * 128:(ob + 1) * 128],
                            outn[:, hp, ncols],
                            start=(hp == 0), stop=(hp == 1))
                    if ob % 2 == 0:
                        nc.scalar.copy(out=y_sb[:, ob], in_=ps)
                    else:
                        nc.vector.tensor_copy(out=y_sb[:, ob], in_=ps)
                    if ob % 2 == 1:
                        nc.sync.dma_start(out=yT[nb][:, ob - 1:ob + 1],
                                          in_=y_sb[:, ob - 1:ob + 1])

            done_e = 0
            done_norm = 0
            pending_norm = None

            def emit_d_with_lag(t):
                nonlocal pending_norm, done_norm
                nfn = phase_d(t)
                if pending_norm is not None:
                    pending_norm()
                    done_norm += 1
                pending_norm = nfn

            # interleave one projection block before each of the first four
            # attention tiles and output-projection blocks between the rest,
            # so the PE always has dense matmul work while exp chains drain
            # (emission order is a scheduling hint; deps keep it correct)
            for t in range(NQT):
                if t < NB:
                    phase_b(t)
                emit_d_with_lag(t)
                while done_e < 2 * NB and done_e + 1 < done_norm:
                    phase_e(done_e // 2, done_e % 2)
                    done_e += 1
            if pending_norm is not None:
                pending_norm()
                done_norm += 1
                pending_norm = None
            while done_e < 2 * NB:
                phase_e(done_e // 2, done_e % 2)
                done_e += 1

    nc.compile()
    return nc


# ------------------------------------------------------------------- kernel

def kernel(x, routes, w_qkv, b_qkv, w_out, b_out, _bench=None):
    import ml_dtypes
    bf = ml_dtypes.bfloat16
    x = np.asarray(x, np.float32)
    routes = np.asarray(routes, np.int32)
    w_qkv = np.asarray(w_qkv, np.float32)
    b_qkv = np.asarray(b_qkv, np.float32)
    w_out = np.asarray(w_out, np.float32)
    b_out = np.asarray(b_out, np.float32)

    from concourse.bass_utils import run_bass_kernel_spmd

    perm, segs, mask = _segments_and_mask(routes)

    key = tuple((t, tuple(s)) for t, s in enumerate(segs))
    if key not in _CACHE:
        _CACHE[key] = _build_program(segs)
    nc = _CACHE[key]

    # per-core inputs
    xs = x[:, perm, :]                                  # sorted tokens
    wq = w_qkv.reshape(3, H, D, DIM)
    bq = b_qkv.reshape(3, H, D)
    in_maps = []
    for c in range(NCORES):
        b, h0 = c // 4, HPC * (c % 4)
        # xT[nb][p, ic, col] = x[b, 512*nb+col, 128*ic+p]
        xT = np.ascontiguousarray(
            xs[b].T.reshape(8, 128, NB, 512).transpose(2, 1, 0, 3)).astype(bf)
        # q/k weight blocks: q(hp0) q(hp1) k(hp0) k(hp1); each 128 chans
        wblk, bblk = [], []
        for s in (0, 1):                                # q, k
            for hp in range(2):
                hh = h0 + 2 * hp
                wblk.append(wq[s, hh:hh + 2].reshape(128, DIM))
                bblk.append(bq[s, hh:hh + 2].reshape(128))
        wcat = np.stack(wblk, 0)                        # (4, 128, DIM)
        wqkT = np.ascontiguousarray(
            wcat.reshape(4, 128, 8, 128).transpose(3, 2, 0, 1)
                .reshape(128, 8, 4 * 128)).astype(bf)
        bqk_a = np.ascontiguousarray(np.stack(bblk, 1))  # (128, 4)
        # v weights, token-major: wvT[p, ic, n] = w_v[h0*D+n, 128*ic+p]
        wv_core = wq[2, h0:h0 + HPC].reshape(HPC * D, DIM)
        wvT = np.ascontiguousarray(
            wv_core.T.reshape(8, 128, 256).transpose(1, 0, 2)).astype(bf)
        woT = np.ascontiguousarray(
            w_out[:, h0 * D:(h0 + HPC) * D].T
                 .reshape(2, 128, DIM).transpose(1, 0, 2)).astype(bf)
        in_maps.append({"xT": xT, "wqkT": wqkT, "wvT": wvT, "bqk": bqk_a,
                        "maskT": mask, "woT": woT})

    res = run_bass_kernel_spmd(nc, in_maps, core_ids=list(range(NCORES)),
                               **(_bench or {}))

    # y partials: yT[nb][p, ob, col] = y[512*nb+col, 128*ob+p]
    y = np.zeros((B, N, DIM), np.float32)
    ys = np.zeros((B, N, DIM), np.float32)
    for c in range(NCORES):
        b = c // 4
        yc = res.results[c]["yT"].astype(np.float32)     # (NB,128,8,512)
        ys[b] += yc.transpose(0, 3, 2, 1).reshape(N, DIM)
    # b_out plus the exactly-foldable v-bias term (sum_k softmax = 1)
    bias = b_out + w_out @ b_qkv[2 * DIM:]
    for b in range(B):
        y[b, perm, :] = ys[b] + bias[None, :]
    if _bench is not None:
        kernel._last = res
    return y
